# revision 12
# baseline (speedup 1.0000x reference)
"""Min-Euclidean-distance retrieval kernel for Trainium2 (8 NeuronCores).

Reference computation:
    x: [1, 2048, 512], y: [1, 65536, 512] (fp32)
    sq[p, r] = ||x_p||^2 + ||y_r||^2 - 2 <x_p, y_r>
    out = min over (p, r) of sqrt(max(sq, 0))

Sharding: candidate pool (R) split across 8 cores, 8192 candidates each.
Host pre-arranges both GEMM operands partition-major in fp8 so each DMA
moves contiguous per-partition runs and the contraction dim lands on SBUF
partitions with no on-chip transposes.

Per core the hot loop is 64 candidate tiles of [128 cand x 2048 queries].
The epilogue (y2 bias + running min over tiles) exceeds what ScalarE
alone can sustain (1.97us/tile vs the PE's 1.73us tile period), so query
columns are split 1536/512 into disjoint PSUM pools / h tiles (Tile's
hazard tracking is tile-granular — shared tiles serialize engines):
  TensorE:  8 fp8 DoubleRow MMs; chunks c0-c2 -> pt_a, c3 -> pt_b
  ScalarE:  h_a = -2*pt_a + y2[r]   (1536 cols; pt_a done at MM#7, so
            this starts one MM early and its 2-period dependency chain
            stays under the PE period)
  VectorE:  h_b = -2*pt_b + y2[r] (tensor_scalar, 512 cols, 0.74us)
            acc_a = min(acc_a, h_a)  (fp16 2x tensor_tensor, 0.96us)
  DMA:      h_b tiles stream raw to DRAM (gpsimd SWDGE queue, 128KB per
            tile ~= 70 GB/s); the host takes the min over those.
The per-query ||x_p||^2 term commutes with the min over candidates and
is added on the host, with the final min across lanes/cores/tiles and
the (monotone) sqrt. fp8 GEMM + fp16 epilogue measure ~1.8e-3 relative
error on the final distance, well inside the 2e-2 tolerance.
"""

import os
import sys

# Recover automatically if a previous process left the NeuronCores wedged.
os.environ.setdefault("NEURON_RT_RESET_CORES", "1")

for _p in ("/opt/trn_rl_repo", "/root/.axon_site/_ro/trn_rl_repo"):
    if _p not in sys.path:
        sys.path.append(_p)

import ml_dtypes
import numpy as np

import concourse.bass as bass
import concourse.mybir as mybir
import concourse.tile as tile
from concourse import bacc, bass_utils

P = 2048          # queries
R = 65536         # candidates (full)
D = 512           # feature dim
NCORES = 8
R_LOC = R // NCORES      # 8192 candidates per core
P_CHUNKS = P // 512      # 4 query chunks (one PSUM bank each)
R_TILES = R_LOC // 128   # 64 candidate tiles
K_TILES = D // 128       # 4 contraction tiles (2 DoubleRow passes)
PA = 1536                # query cols on the ScalarE/VectorE-min path
PB = P - PA              # query cols shipped raw to the host
# Mid-stream tiles whose h_a ships raw to DRAM (host takes their min):
# every 4th tile from 17, when the input-DMA rings have gone idle.
SHIP_A = []  # h_a shipping disabled (v8)

F32 = mybir.dt.float32
MM_DT = mybir.dt.float8e4
MM_NP = ml_dtypes.float8_e4m3
ACC_DT, ACC_NP = mybir.dt.float16, np.float16
# The epilogue runs in fp16. A constant shift keeps the values that matter
# (near the global min, sq ~ 650 => h ~ 150) small; fp16 quantum there is
# ~0.125, negligible next to the fp8 GEMM noise.
Y2_SHIFT = np.float32(512.0)


def _build_module() -> bass.Bass:
    nc = bacc.Bacc("TRN2", target_bir_lowering=False, debug=False)

    # Host-prepared layouts (partition-major, contiguous per partition):
    #   xt[q, c, k, j]  = x[c*512 + j, k*128 + q]
    #   yt[q, t, k, s]  = y[t*128 + s, k*128 + q]   (t-major: one candidate
    #                     tile = one contiguous 512B-per-partition slice)
    #   y2t[lane, t]    = ||y_r||^2 - Y2_SHIFT for r = t*128 + lane
    xt = nc.dram_tensor("xt", [128, P_CHUNKS, K_TILES, 512], MM_DT,
                        kind="ExternalInput")
    yt = nc.dram_tensor("yt", [128, R_TILES, K_TILES, 128], MM_DT,
                        kind="ExternalInput")
    y2t = nc.dram_tensor("y2t", [128, R_TILES], F32, kind="ExternalInput")
    # out[lane, p<PA] = min over r-tiles t of (y2[t*128+lane] - 2 G[.])
    out = nc.dram_tensor("out", [128, PA], ACC_DT, kind="ExternalOutput")
    # hbd[lane, t, j] = y2[t*128+lane] - 2 G[t*128+lane, PA+j]  (no min)
    hbd = nc.dram_tensor("hbd", [128, R_TILES, PB], ACC_DT,
                         kind="ExternalOutput")
    # Raw h_a tiles for SHIP_A tiles (min taken on host) — removes their
    # tensor_tensor from VectorE, whose TS+TT load otherwise exceeds the
    # PE tile period.
    had = (nc.dram_tensor("had", [128, len(SHIP_A), PA], ACC_DT,
                          kind="ExternalOutput") if SHIP_A else None)

    with tile.TileContext(nc) as tc:
        with (
            tc.tile_pool(name="big", bufs=1) as big,
            tc.tile_pool(name="scra", bufs=8) as scra,
            tc.tile_pool(name="scrb", bufs=16) as scrb,
            tc.tile_pool(name="psa", bufs=2, space="PSUM") as psa,
            tc.tile_pool(name="psb", bufs=2, space="PSUM") as psb,
        ):
            xt_sb = big.tile([128, P_CHUNKS, K_TILES, 512], MM_DT)
            yt_sb = big.tile([128, R_TILES, K_TILES, 128], MM_DT)
            y2t_sb = big.tile([128, R_TILES], F32)
            acc_a = big.tile([128, PA], ACC_DT)

            # Leading-edge DMAs: sync (SP) + scalar (Activation) HWDGE rings
            # (~60 GB/s each) carry all of x (needed within two tiles)
            # interleaved in MM-consumption order plus the first y tile;
            # gpsimd SWDGE delivers y2t + y tiles 1-3. Bulk y prefetch is
            # spread over all three queues, staying ahead of the PE.
            nc.sync.dma_start(yt_sb[:, 0], yt.ap()[:, 0])
            for kk in (0, 1):
                nc.scalar.dma_start(xt_sb[:, 0, 2 * kk : 2 * kk + 2],
                                    xt.ap()[:, 0, 2 * kk : 2 * kk + 2])
                nc.sync.dma_start(xt_sb[:, 1, 2 * kk : 2 * kk + 2],
                                  xt.ap()[:, 1, 2 * kk : 2 * kk + 2])
                nc.scalar.dma_start(xt_sb[:, 2, 2 * kk : 2 * kk + 2],
                                    xt.ap()[:, 2, 2 * kk : 2 * kk + 2])
                nc.sync.dma_start(xt_sb[:, 3, 2 * kk : 2 * kk + 2],
                                  xt.ap()[:, 3, 2 * kk : 2 * kk + 2])
            nc.gpsimd.dma_start(y2t_sb[:], y2t.ap())
            for t in range(1, 8):
                nc.gpsimd.dma_start(yt_sb[:, t], yt.ap()[:, t])
            for t4 in range(2, 4):
                nc.sync.dma_start(yt_sb[:, 4 * t4 : 4 * t4 + 4],
                                  yt.ap()[:, 4 * t4 : 4 * t4 + 4])
            for t4 in range(4, 8):
                nc.scalar.dma_start(yt_sb[:, 4 * t4 : 4 * t4 + 4],
                                    yt.ap()[:, 4 * t4 : 4 * t4 + 4])
            for t4 in range(8, 16):
                nc.gpsimd.dma_start(yt_sb[:, 4 * t4 : 4 * t4 + 4],
                                    yt.ap()[:, 4 * t4 : 4 * t4 + 4])

            prev_ha = None
            for t in range(R_TILES):
                pa = psa.tile([128, PA], F32, name="pa")
                pb = psb.tile([128, PB], F32, name="pb")
                # kk outer keeps the stationary operand loaded across
                # chunks; c3 last so pt_a completes at MM#7 and ScalarE
                # starts one MM early.
                for kk in range(K_TILES // 2):
                    for c in range(P_CHUNKS):
                        dst = (pa[:, c * 512 : (c + 1) * 512]
                               if c < 3 else pb[:])
                        nc.tensor.matmul(
                            dst,
                            lhsT=yt_sb[:, t, 2 * kk : 2 * kk + 2],
                            rhs=xt_sb[:, c, 2 * kk : 2 * kk + 2],
                            start=(kk == 0),
                            stop=(kk == K_TILES // 2 - 1),
                            perf_mode=mybir.MatmulPerfMode.DoubleRow,
                        )
                bias = y2t_sb[:, t : t + 1]
                ha = acc_a if t == 0 else scra.tile([128, PA], ACC_DT, name="ha")
                hb = scrb.tile([128, PB], ACC_DT, name="hb")
                nc.scalar.activation(
                    out=ha[:],
                    in_=pa[:],
                    func=mybir.ActivationFunctionType.Identity,
                    bias=bias,
                    scale=-2.0,
                )
                nc.vector.tensor_scalar(
                    out=hb[:],
                    in0=pb[:],
                    scalar1=-2.0,
                    scalar2=bias,
                    op0=mybir.AluOpType.mult,
                    op1=mybir.AluOpType.add,
                )
                # hbd transfers serialize at ~2us/128KB per queue; split
                # even/odd across the SWDGE and the (post-input-idle) sync
                # ring so the drain finishes well before the compute does.
                # Tiles >=48 all ride sync: the exit sequence pays a ~5us
                # GpSimd DRAIN if SWDGE transfers are still in flight, and
                # late-tile transfers gate on tensor_scalar anyway.
                if t >= 48:
                    hb_eng = nc.sync
                else:
                    hb_eng = nc.gpsimd if t % 2 == 0 else nc.sync
                hb_eng.dma_start(hbd.ap()[:, t], hb[:])
                if t in SHIP_A:
                    eng = nc.sync if (t // 4) % 2 == 0 else nc.gpsimd
                    eng.dma_start(had.ap()[:, SHIP_A.index(t)], ha[:])
                # The min runs one tile late so VectorE's tensor_scalar
                # (which frees the psb buffer) never queues behind it.
                if prev_ha is not None:
                    nc.vector.tensor_tensor(
                        out=acc_a[:], in0=acc_a[:], in1=prev_ha[:],
                        op=mybir.AluOpType.min,
                    )
                    prev_ha = None
                if t != 0 and t not in SHIP_A:
                    prev_ha = ha
            nc.vector.tensor_tensor(
                out=acc_a[:], in0=acc_a[:], in1=prev_ha[:], op=mybir.AluOpType.min,
            )
            nc.sync.dma_start(out.ap()[:, 0:768], acc_a[:, 0:768])
            nc.scalar.dma_start(out.ap()[:, 768:PA], acc_a[:, 768:PA])
    nc.compile()
    return nc


_module_cache: bass.Bass | None = None


def _get_module() -> bass.Bass:
    global _module_cache
    if _module_cache is None:
        _module_cache = _build_module()
    return _module_cache


def _prepare_inputs(x: np.ndarray, y: np.ndarray):
    """Host-side sharding/layout prep. Returns per-core input maps."""
    # xt[q, c, k, j] = x[c*512 + j, k*128 + q]
    xt4 = x.T.reshape(K_TILES, 128, P_CHUNKS, 512)
    xt = np.ascontiguousarray(xt4.transpose(1, 2, 0, 3).astype(MM_NP))
    in_maps = []
    for cc in range(NCORES):
        yc = y[cc * R_LOC : (cc + 1) * R_LOC]
        # yt[q, t, k, s] = yc[t*128 + s, k*128 + q]
        a = yc.reshape(R_TILES, 128, K_TILES, 128)
        yct = np.ascontiguousarray(a.transpose(3, 0, 2, 1).astype(MM_NP))
        y2 = np.einsum("rd,rd->r", yc, yc, dtype=np.float32) - Y2_SHIFT
        y2t = np.ascontiguousarray(y2.reshape(R_TILES, 128).T)
        in_maps.append({"xt": xt, "yt": yct, "y2t": y2t})
    return in_maps


def _postprocess(x: np.ndarray, accs: np.ndarray, hbds: np.ndarray,
                 hads: np.ndarray) -> np.ndarray:
    """accs: [NCORES, 128, PA]; hbds: [NCORES, 128, T, PB]; hads raw."""
    x2 = np.einsum("pd,pd->p", x, x, dtype=np.float32)
    ma = accs.astype(np.float32).min(axis=(0, 1))
    if hads is not None:
        ma = np.minimum(ma, hads.astype(np.float32).min(axis=(0, 1, 2)))
    ma = ma + Y2_SHIFT
    mb = hbds.astype(np.float32).min(axis=(0, 1, 2)) + Y2_SHIFT  # [PB]
    m = np.concatenate([ma, mb])
    sq_min = np.float32((x2 + m).min())
    return np.sqrt(np.maximum(sq_min, np.float32(0.0)), dtype=np.float32)


def kernel(
    predicted_transaction_company: np.ndarray,
    future_transaction_companies_inc_current_data: np.ndarray,
) -> np.ndarray:
    x = np.asarray(predicted_transaction_company, dtype=np.float32)[0]
    y = np.asarray(future_transaction_companies_inc_current_data, dtype=np.float32)[0]

    nc = _get_module()
    in_maps = _prepare_inputs(x, y)
    res = bass_utils.run_bass_kernel_spmd(nc, in_maps, core_ids=list(range(NCORES)))
    accs = np.stack([r["out"] for r in res.results])
    hbds = np.stack([r["hbd"] for r in res.results])
    hads = (np.stack([r["had"] for r in res.results])
            if SHIP_A else None)
    return _postprocess(x, accs, hbds, hads)


# revision 13
# speedup vs baseline: 1.0078x; 1.0078x over previous
"""Min-Euclidean-distance retrieval kernel for Trainium2 (8 NeuronCores).

Reference computation:
    x: [1, 2048, 512], y: [1, 65536, 512] (fp32)
    sq[p, r] = ||x_p||^2 + ||y_r||^2 - 2 <x_p, y_r>
    out = min over (p, r) of sqrt(max(sq, 0))

Sharding: candidate pool (R) split across 8 cores, 8192 candidates each.
Host pre-arranges both GEMM operands partition-major in fp8 so each DMA
moves contiguous per-partition runs and the contraction dim lands on SBUF
partitions with no on-chip transposes.

Per core the hot loop is 64 candidate tiles of [128 cand x 2048 queries].
The epilogue (y2 bias + running min over tiles) exceeds what ScalarE
alone can sustain (1.97us/tile vs the PE's 1.73us tile period), so query
columns are split 1536/512 into disjoint PSUM pools / h tiles (Tile's
hazard tracking is tile-granular — shared tiles serialize engines):
  TensorE:  8 fp8 DoubleRow MMs; chunks c0-c2 -> pt_a, c3 -> pt_b
  ScalarE:  h_a = -2*pt_a + y2[r]   (1536 cols; pt_a done at MM#7, so
            this starts one MM early and its 2-period dependency chain
            stays under the PE period)
  VectorE:  h_b = -2*pt_b + y2[r] (tensor_scalar, 512 cols, 0.74us)
            acc_a = min(acc_a, h_a)  (fp16 2x tensor_tensor, 0.96us)
  DMA:      h_b tiles stream raw to DRAM (gpsimd SWDGE queue, 128KB per
            tile ~= 70 GB/s); the host takes the min over those.
The per-query ||x_p||^2 term commutes with the min over candidates and
is added on the host, with the final min across lanes/cores/tiles and
the (monotone) sqrt. fp8 GEMM + fp16 epilogue measure ~1.8e-3 relative
error on the final distance, well inside the 2e-2 tolerance.
"""

import os
import sys

# Recover automatically if a previous process left the NeuronCores wedged.
os.environ.setdefault("NEURON_RT_RESET_CORES", "1")

for _p in ("/opt/trn_rl_repo", "/root/.axon_site/_ro/trn_rl_repo"):
    if _p not in sys.path:
        sys.path.append(_p)

import ml_dtypes
import numpy as np

import concourse.bass as bass
import concourse.mybir as mybir
import concourse.tile as tile
from concourse import bacc, bass_utils

P = 2048          # queries
R = 65536         # candidates (full)
D = 512           # feature dim
NCORES = 8
R_LOC = R // NCORES      # 8192 candidates per core
P_CHUNKS = P // 512      # 4 query chunks (one PSUM bank each)
R_TILES = R_LOC // 128   # 64 candidate tiles
K_TILES = D // 128       # 4 contraction tiles (2 DoubleRow passes)
PA = 1536                # query cols on the ScalarE/VectorE-min path
PB = P - PA              # query cols shipped raw to the host
# Mid-stream tiles whose h_a ships raw to DRAM (host takes their min):
# every 4th tile from 17, when the input-DMA rings have gone idle.
SHIP_A = []  # h_a shipping disabled (v8)

F32 = mybir.dt.float32
MM_DT = mybir.dt.float8e4
MM_NP = ml_dtypes.float8_e4m3
ACC_DT, ACC_NP = mybir.dt.float16, np.float16
# The epilogue runs in fp16. A constant shift keeps the values that matter
# (near the global min, sq ~ 650 => h ~ 150) small; fp16 quantum there is
# ~0.125, negligible next to the fp8 GEMM noise.
Y2_SHIFT = np.float32(512.0)


def _build_module() -> bass.Bass:
    nc = bacc.Bacc("TRN2", target_bir_lowering=False, debug=False)

    # Host-prepared layouts (partition-major, contiguous per partition):
    #   xt[q, c, k, j]  = x[c*512 + j, k*128 + q]
    #   yt[q, t, k, s]  = y[t*128 + s, k*128 + q]   (t-major: one candidate
    #                     tile = one contiguous 512B-per-partition slice)
    #   y2t[lane, t]    = ||y_r||^2 - Y2_SHIFT for r = t*128 + lane
    xt = nc.dram_tensor("xt", [128, P_CHUNKS, K_TILES, 512], MM_DT,
                        kind="ExternalInput")
    yt = nc.dram_tensor("yt", [128, R_TILES, K_TILES, 128], MM_DT,
                        kind="ExternalInput")
    y2t = nc.dram_tensor("y2t", [128, R_TILES], F32, kind="ExternalInput")
    # out[lane, p<PA] = min over r-tiles t of (y2[t*128+lane] - 2 G[.])
    out = nc.dram_tensor("out", [128, PA], ACC_DT, kind="ExternalOutput")
    # hbd[lane, t, j] = y2[t*128+lane] - 2 G[t*128+lane, PA+j]  (no min)
    hbd = nc.dram_tensor("hbd", [128, R_TILES, PB], ACC_DT,
                         kind="ExternalOutput")
    # Raw h_a tiles for SHIP_A tiles (min taken on host) — removes their
    # tensor_tensor from VectorE, whose TS+TT load otherwise exceeds the
    # PE tile period.
    had = (nc.dram_tensor("had", [128, len(SHIP_A), PA], ACC_DT,
                          kind="ExternalOutput") if SHIP_A else None)

    with tile.TileContext(nc) as tc:
        with (
            tc.tile_pool(name="big", bufs=1) as big,
            tc.tile_pool(name="scra", bufs=6) as scra,
            tc.tile_pool(name="scrb", bufs=12) as scrb,
            tc.tile_pool(name="psa", bufs=2, space="PSUM") as psa,
            tc.tile_pool(name="psb", bufs=2, space="PSUM") as psb,
        ):
            xt_sb = big.tile([128, P_CHUNKS, K_TILES, 512], MM_DT)
            yt_sb = big.tile([128, R_TILES, K_TILES, 128], MM_DT)
            y2t_sb = big.tile([128, R_TILES], F32)
            acc_a = big.tile([128, PA], ACC_DT)

            # Leading-edge DMAs: sync (SP) + scalar (Activation) HWDGE rings
            # (~60 GB/s each) carry all of x (needed within two tiles)
            # interleaved in MM-consumption order plus the first y tile;
            # gpsimd SWDGE delivers y2t + y tiles 1-3. Bulk y prefetch is
            # spread over all three queues, staying ahead of the PE.
            nc.sync.dma_start(yt_sb[:, 0], yt.ap()[:, 0])
            for kk in (0, 1):
                nc.scalar.dma_start(xt_sb[:, 0, 2 * kk : 2 * kk + 2],
                                    xt.ap()[:, 0, 2 * kk : 2 * kk + 2])
                nc.scalar.dma_start(xt_sb[:, 2, 2 * kk : 2 * kk + 2],
                                    xt.ap()[:, 2, 2 * kk : 2 * kk + 2])
            for c in (1, 3):
                nc.sync.dma_start(xt_sb[:, c, 0:2], xt.ap()[:, c, 0:2])
                nc.gpsimd.dma_start(xt_sb[:, c, 2:4], xt.ap()[:, c, 2:4])
            nc.gpsimd.dma_start(y2t_sb[:], y2t.ap())
            for t in range(1, 8):
                nc.gpsimd.dma_start(yt_sb[:, t], yt.ap()[:, t])
            for t4 in range(2, 4):
                nc.sync.dma_start(yt_sb[:, 4 * t4 : 4 * t4 + 4],
                                  yt.ap()[:, 4 * t4 : 4 * t4 + 4])
            for t4 in range(4, 8):
                nc.scalar.dma_start(yt_sb[:, 4 * t4 : 4 * t4 + 4],
                                    yt.ap()[:, 4 * t4 : 4 * t4 + 4])
            for t4 in range(8, 16):
                nc.gpsimd.dma_start(yt_sb[:, 4 * t4 : 4 * t4 + 4],
                                    yt.ap()[:, 4 * t4 : 4 * t4 + 4])

            prev_ha = None
            for t in range(R_TILES):
                pa = psa.tile([128, PA], F32, name="pa")
                pb = psb.tile([128, PB], F32, name="pb")
                # kk outer keeps the stationary operand loaded across
                # chunks; c3 last so pt_a completes at MM#7 and ScalarE
                # starts one MM early.
                for kk in range(K_TILES // 2):
                    for c in range(P_CHUNKS):
                        dst = (pa[:, c * 512 : (c + 1) * 512]
                               if c < 3 else pb[:])
                        nc.tensor.matmul(
                            dst,
                            lhsT=yt_sb[:, t, 2 * kk : 2 * kk + 2],
                            rhs=xt_sb[:, c, 2 * kk : 2 * kk + 2],
                            start=(kk == 0),
                            stop=(kk == K_TILES // 2 - 1),
                            perf_mode=mybir.MatmulPerfMode.DoubleRow,
                        )
                bias = y2t_sb[:, t : t + 1]
                ha = acc_a if t == 0 else scra.tile([128, PA], ACC_DT, name="ha")
                hb = scrb.tile([128, PB], ACC_DT, name="hb")
                nc.scalar.activation(
                    out=ha[:],
                    in_=pa[:],
                    func=mybir.ActivationFunctionType.Identity,
                    bias=bias,
                    scale=-2.0,
                )
                nc.vector.tensor_scalar(
                    out=hb[:],
                    in0=pb[:],
                    scalar1=-2.0,
                    scalar2=bias,
                    op0=mybir.AluOpType.mult,
                    op1=mybir.AluOpType.add,
                )
                # hbd transfers serialize at ~2us/128KB per queue; split
                # even/odd across the SWDGE and the (post-input-idle) sync
                # ring so the drain finishes well before the compute does.
                # Tiles >=48 all ride sync: the exit sequence pays a ~5us
                # GpSimd DRAIN if SWDGE transfers are still in flight, and
                # late-tile transfers gate on tensor_scalar anyway.
                if t == R_TILES - 1:
                    nc.sync.dma_start(hbd.ap()[:, t, 0:256], hb[:, 0:256])
                    nc.scalar.dma_start(hbd.ap()[:, t, 256:PB], hb[:, 256:PB])
                else:
                    if t >= 48:
                        hb_eng = nc.sync
                    else:
                        hb_eng = nc.gpsimd if t % 2 == 0 else nc.sync
                    hb_eng.dma_start(hbd.ap()[:, t], hb[:])
                if t in SHIP_A:
                    eng = nc.sync if (t // 4) % 2 == 0 else nc.gpsimd
                    eng.dma_start(had.ap()[:, SHIP_A.index(t)], ha[:])
                # The min runs one tile late so VectorE's tensor_scalar
                # (which frees the psb buffer) never queues behind it.
                if prev_ha is not None:
                    nc.vector.tensor_tensor(
                        out=acc_a[:], in0=acc_a[:], in1=prev_ha[:],
                        op=mybir.AluOpType.min,
                    )
                    prev_ha = None
                if t != 0 and t not in SHIP_A:
                    prev_ha = ha
            nc.vector.tensor_tensor(
                out=acc_a[:], in0=acc_a[:], in1=prev_ha[:], op=mybir.AluOpType.min,
            )
            nc.sync.dma_start(out.ap()[:, 0:768], acc_a[:, 0:768])
            nc.scalar.dma_start(out.ap()[:, 768:PA], acc_a[:, 768:PA])
    nc.compile()
    return nc


_module_cache: bass.Bass | None = None


def _get_module() -> bass.Bass:
    global _module_cache
    if _module_cache is None:
        _module_cache = _build_module()
    return _module_cache


def _prepare_inputs(x: np.ndarray, y: np.ndarray):
    """Host-side sharding/layout prep. Returns per-core input maps."""
    # xt[q, c, k, j] = x[c*512 + j, k*128 + q]
    xt4 = x.T.reshape(K_TILES, 128, P_CHUNKS, 512)
    xt = np.ascontiguousarray(xt4.transpose(1, 2, 0, 3).astype(MM_NP))
    in_maps = []
    for cc in range(NCORES):
        yc = y[cc * R_LOC : (cc + 1) * R_LOC]
        # yt[q, t, k, s] = yc[t*128 + s, k*128 + q]
        a = yc.reshape(R_TILES, 128, K_TILES, 128)
        yct = np.ascontiguousarray(a.transpose(3, 0, 2, 1).astype(MM_NP))
        y2 = np.einsum("rd,rd->r", yc, yc, dtype=np.float32) - Y2_SHIFT
        y2t = np.ascontiguousarray(y2.reshape(R_TILES, 128).T)
        in_maps.append({"xt": xt, "yt": yct, "y2t": y2t})
    return in_maps


def _postprocess(x: np.ndarray, accs: np.ndarray, hbds: np.ndarray,
                 hads: np.ndarray) -> np.ndarray:
    """accs: [NCORES, 128, PA]; hbds: [NCORES, 128, T, PB]; hads raw."""
    x2 = np.einsum("pd,pd->p", x, x, dtype=np.float32)
    ma = accs.astype(np.float32).min(axis=(0, 1))
    if hads is not None:
        ma = np.minimum(ma, hads.astype(np.float32).min(axis=(0, 1, 2)))
    ma = ma + Y2_SHIFT
    mb = hbds.astype(np.float32).min(axis=(0, 1, 2)) + Y2_SHIFT  # [PB]
    m = np.concatenate([ma, mb])
    sq_min = np.float32((x2 + m).min())
    return np.sqrt(np.maximum(sq_min, np.float32(0.0)), dtype=np.float32)


def kernel(
    predicted_transaction_company: np.ndarray,
    future_transaction_companies_inc_current_data: np.ndarray,
) -> np.ndarray:
    x = np.asarray(predicted_transaction_company, dtype=np.float32)[0]
    y = np.asarray(future_transaction_companies_inc_current_data, dtype=np.float32)[0]

    nc = _get_module()
    in_maps = _prepare_inputs(x, y)
    res = bass_utils.run_bass_kernel_spmd(nc, in_maps, core_ids=list(range(NCORES)))
    accs = np.stack([r["out"] for r in res.results])
    hbds = np.stack([r["hbd"] for r in res.results])
    hads = (np.stack([r["had"] for r in res.results])
            if SHIP_A else None)
    return _postprocess(x, accs, hbds, hads)
